# revision 8
# baseline (speedup 1.0000x reference)
"""GPT2 attention (B=2,S=2048,D=1024,H=16,hd=64, no causal mask) on 8 trn2 cores.

Sharding: core c handles batch b=c//4 and head-group g=c%4 (4 heads).
w_attn columns are split per head group (Q scaled by 1/sqrt(hd) on host);
w_proj rows split per head group; host sums the 4 partial c_proj outputs
per batch (the "all-reduce").

Per-core dataflow (matmuls in float32r, 1 cyc/row at N>=512; every tile a
matmul consumes is written as float32r by its producer so walrus' rounding
check passes):
  hid [2048,1024] --PE transpose--> hidT [1024,2048]
  qkvT[768,2048] = w_slice.T @ hidT   (feature-major Q^T,K^T,V^T, 2 heads/tile)
  V^T --PE transpose--> vaug [k,65] tiles (col 64 = ones for denominator)
  per (head, 512-wide q chunk):
    S^T[k,q] tiles = K^T_tile.T @ Q^T  -> DVE copy to SBUF block [128, 4096]
    one ACT exp per block (amortizes ACT fixed cost; no max-subtraction:
    scores are O(1) so exp is numerically safe)
    O_u^T[65,512] = sum_k vaug.T @ E   (row 64 = softmax denominator)
    obar_h = O_u^T[0:64] * broadcast(1/denom)  (ones-matmul broadcast + DVE mul)
  out[q,1024] = sum_h obar_h.T @ wp_h  (K=64 accumulation, 4 heads)
"""

import sys

import numpy as np

if "/opt/trn_rl_repo" not in sys.path:
    sys.path.insert(0, "/opt/trn_rl_repo")

S = 2048
D = 1024
P = 128
NH = 4  # heads per core
HD = 64
N_CORES = 8

_CACHE = {}


def _build_program():
    import concourse.mybir as mybir
    from concourse import bacc
    from concourse.masks import make_identity
    from concourse.tile import TileContext

    f32r = mybir.dt.float32r
    f32 = mybir.dt.float32
    AF = mybir.ActivationFunctionType
    ALU = mybir.AluOpType

    nc = bacc.Bacc(None, target_bir_lowering=False, debug=False)
    hid = nc.declare_dram_parameter("hid", [S, D], f32r, isOutput=False)
    wqkv = nc.declare_dram_parameter("wqkv", [D, 3 * NH * HD], f32r, isOutput=False)
    wp = nc.declare_dram_parameter("wp", [NH * HD, D], f32r, isOutput=False)
    out = nc.declare_dram_parameter("out", [S, D], f32, isOutput=True)

    with TileContext(nc) as tc:
        with tc.tile_pool(name="const", bufs=1) as constp:
            ident_f = constp.tile([P, P], f32)
            make_identity(nc, ident_f)
            ident = constp.tile([P, P], f32r)
            nc.vector.tensor_copy(ident[:], ident_f[:])
            ones_f = constp.tile([P, HD], f32)
            nc.gpsimd.memset(ones_f[:], 1.0)
            ones_t = constp.tile([P, HD], f32r)
            nc.vector.tensor_copy(ones_t[:], ones_f[:])

            qkvT = [constp.tile([P, S], f32r, name=f"qkvT{i}") for i in range(6)]
            vaug = constp.tile([P, NH * 16 * 65], f32r)

            # ---------------- Stage A: hidT + QKV ----------------
            with tc.tile_pool(name="hidT_pool", bufs=1) as hidTp, \
                 tc.tile_pool(name="stageA", bufs=3) as sA, \
                 tc.tile_pool(name="w_pool", bufs=1) as wpool, \
                 tc.tile_pool(name="tpsum", bufs=3, space="PSUM") as tpsum, \
                 tc.tile_pool(name="qpsum", bufs=3, space="PSUM") as qpsum:
                hidT = [hidTp.tile([P, S], f32r, name=f"hidT{i}") for i in range(8)]
                w_sb = [wpool.tile([P, 768], f32r, name=f"w{i}") for i in range(8)]
                for i in range(8):
                    nc.sync.dma_start(out=w_sb[i][:], in_=wqkv[i * P : (i + 1) * P, :])
                for st in range(16):
                    ht = sA.tile([P, D], f32r, tag="hidload")
                    nc.sync.dma_start(out=ht[:], in_=hid[st * P : (st + 1) * P, :])
                    for dt_ in range(8):
                        tp = tpsum.tile([P, P], f32r, tag="tp")
                        nc.tensor.transpose(
                            tp[:], ht[:, dt_ * P : (dt_ + 1) * P], ident[:]
                        )
                        nc.vector.tensor_copy(
                            hidT[dt_][:, st * P : (st + 1) * P], tp[:]
                        )
                for ct in range(6):
                    for qc in range(4):
                        ps = qpsum.tile([P, 512], f32, tag="qkvps")
                        for dt_ in range(8):
                            nc.tensor.matmul(
                                ps[:],
                                lhsT=w_sb[dt_][:, ct * P : (ct + 1) * P],
                                rhs=hidT[dt_][:, qc * 512 : (qc + 1) * 512],
                                start=(dt_ == 0),
                                stop=(dt_ == 7),
                            )
                        nc.vector.tensor_copy(
                            qkvT[ct][:, qc * 512 : (qc + 1) * 512], ps[:]
                        )
                # V seq-major (transpose V^T) into vaug; col 64 of each 65 = ones
                for h in range(NH):
                    par = HD * (h % 2)
                    vsrc = qkvT[4 + h // 2]
                    for kt in range(16):
                        vp = tpsum.tile([P, P], f32r, tag="tp")
                        nc.tensor.transpose(
                            vp[:, :HD],
                            vsrc[par : par + HD, kt * P : (kt + 1) * P],
                            ident[par : par + HD, par : par + HD],
                        )
                        base = (h * 16 + kt) * 65
                        nc.vector.tensor_copy(vaug[:, base : base + HD], vp[:, :HD])
                        nc.vector.tensor_copy(
                            vaug[:, base + HD : base + 65], ones_f[:, 0:1]
                        )

            # ---------------- Stages B+C ----------------
            with tc.tile_pool(name="persistBC", bufs=1) as perBC:
                obar = [perBC.tile([HD, S], f32r, name=f"obar{i}") for i in range(NH)]
                wp_sb = [perBC.tile([HD, D], f32r, name=f"wp{i}") for i in range(NH)]
                for h in range(NH):
                    nc.sync.dma_start(
                        out=wp_sb[h][:], in_=wp[h * HD : (h + 1) * HD, :]
                    )

                with tc.tile_pool(name="sblk", bufs=2) as sblk, \
                     tc.tile_pool(name="npool", bufs=3) as npool, \
                     tc.tile_pool(name="spsum", bufs=4, space="PSUM") as spsum, \
                     tc.tile_pool(name="opsum", bufs=2, space="PSUM") as opsum, \
                     tc.tile_pool(name="rpsum", bufs=2, space="PSUM") as rpsum:
                    for h in range(NH):
                        par = HD * (h % 2)
                        qT = qkvT[0 + h // 2]
                        kT = qkvT[2 + h // 2]
                        for qc in range(4):
                            op = opsum.tile([65, 512], f32, tag="op")
                            for half in range(2):
                                sb = sblk.tile([P, 8 * 512], f32r, tag="sb")
                                for j in range(8):
                                    kt = half * 8 + j
                                    sp = spsum.tile([P, 512], f32, tag="sp")
                                    nc.tensor.matmul(
                                        sp[:],
                                        lhsT=kT[par : par + HD, kt * P : (kt + 1) * P],
                                        rhs=qT[par : par + HD, qc * 512 : (qc + 1) * 512],
                                        start=True,
                                        stop=True,
                                    )
                                    nc.vector.tensor_copy(
                                        sb[:, j * 512 : (j + 1) * 512], sp[:]
                                    )
                                nc.scalar.activation(sb[:], sb[:], AF.Exp)
                                for j in range(8):
                                    kt = half * 8 + j
                                    base = (h * 16 + kt) * 65
                                    nc.tensor.matmul(
                                        op[:],
                                        lhsT=vaug[:, base : base + 65],
                                        rhs=sb[:, j * 512 : (j + 1) * 512],
                                        start=(kt == 0),
                                        stop=(kt == 15),
                                    )
                            rec = npool.tile([P, 512], f32r, tag="rec")
                            with nc.allow_low_precision(
                                reason="f32r recip of softmax denom"
                            ):
                                nc.vector.reciprocal(rec[64:65, :], op[64:65, :])
                            rb = rpsum.tile([HD, 512], f32, tag="rb")
                            nc.tensor.matmul(
                                rb[:], lhsT=ones_t[64:65, :], rhs=rec[64:65, :],
                                start=True, stop=True,
                            )
                            ou_sb = npool.tile([HD, 512], f32r, tag="ou")
                            nc.vector.tensor_copy(ou_sb[:], op[0:HD, :])
                            rb_sb = npool.tile([HD, 512], f32r, tag="rbs")
                            nc.vector.tensor_copy(rb_sb[:], rb[:])
                            with nc.allow_low_precision(
                                reason="softmax normalize in f32r"
                            ):
                                nc.vector.tensor_tensor(
                                    out=obar[h][:, qc * 512 : (qc + 1) * 512],
                                    in0=ou_sb[:],
                                    in1=rb_sb[:],
                                    op=ALU.mult,
                                )

                # ---------------- Stage C: projection ----------------
                with tc.tile_pool(name="outp", bufs=4) as outp, \
                     tc.tile_pool(name="ppsum", bufs=4, space="PSUM") as ppsum:
                    for qt in range(16):
                        ot = outp.tile([P, D], f32, tag="ot")
                        for ec in range(2):
                            pp = ppsum.tile([P, 512], f32, tag="pp")
                            for h in range(NH):
                                nc.tensor.matmul(
                                    pp[:],
                                    lhsT=obar[h][:, qt * P : (qt + 1) * P],
                                    rhs=wp_sb[h][:, ec * 512 : (ec + 1) * 512],
                                    start=(h == 0),
                                    stop=(h == NH - 1),
                                )
                            nc.vector.tensor_copy(
                                ot[:, ec * 512 : (ec + 1) * 512], pp[:]
                            )
                        nc.sync.dma_start(
                            out=out[qt * P : (qt + 1) * P, :], in_=ot[:]
                        )

    nc.compile()
    return nc


def _get_nc():
    if "nc" not in _CACHE:
        _CACHE["nc"] = _build_program()
    return _CACHE["nc"]


def _shard_inputs(hidden_states, w_attn, w_proj):
    scale = 1.0 / np.sqrt(np.float32(HD))
    in_maps = []
    for c in range(N_CORES):
        b, g = divmod(c, 4)
        cs = slice(g * NH * HD, (g + 1) * NH * HD)
        wq = w_attn[:, 0:D][:, cs] * scale
        wk = w_attn[:, D : 2 * D][:, cs]
        wv = w_attn[:, 2 * D : 3 * D][:, cs]
        in_maps.append(
            {
                "hid": np.ascontiguousarray(hidden_states[b], dtype=np.float32),
                "wqkv": np.ascontiguousarray(
                    np.concatenate([wq, wk, wv], axis=1), dtype=np.float32
                ),
                "wp": np.ascontiguousarray(w_proj[cs, :], dtype=np.float32),
            }
        )
    return in_maps


def run(hidden_states, w_attn, w_proj, trace=False):
    from concourse.bass_utils import run_bass_kernel_spmd

    nc = _get_nc()
    in_maps = _shard_inputs(hidden_states, w_attn, w_proj)
    res = run_bass_kernel_spmd(nc, in_maps, list(range(N_CORES)), trace=trace)
    parts = [res.results[c]["out"] for c in range(N_CORES)]
    out = np.stack(
        [
            parts[0] + parts[1] + parts[2] + parts[3],
            parts[4] + parts[5] + parts[6] + parts[7],
        ]
    ).astype(np.float32)
    return out, res


def kernel(hidden_states, w_attn, w_proj):
    out, _ = run(
        np.asarray(hidden_states), np.asarray(w_attn), np.asarray(w_proj)
    )
    return out
